# revision 2
# baseline (speedup 1.0000x reference)
"""Two-layer GCN + global mean pool on 8 Trainium2 NeuronCores.

Strategy (dst-sharded message passing, one SPMD program):
- Nodes are range-sharded across the 8 cores (12500 dsts each). Each core
  processes the edges whose dst lies in its shard.
- Symmetric norm is factorized: the gather tables are pre-scaled by
  dinv[src] on the producer side, and dinv[dst] is applied to the
  aggregate on the consumer side, so the per-edge weight never appears.
- Layer 1 aggregates x-space messages (A~ x) then applies W1 (math:
  A~(xW1) == (A~x)W1), so the gather table is just dinv*x.
- Per-edge gathers use the custom SWDGE dma_gather (int16 indices, 4
  table chunks of 25000 rows, <=1024 idxs/call, 4 SWDGE queues).
- Segment-sum is a PE one-hot matmul: out[ch, dst] += msgs[e, ch].T @
  onehot[e, dst] accumulated in pre-zeroed PSUM banks; one-hot blocks are
  generated on DVE by comparing per-edge local dst ids against an iota row.
- Between layers, the per-shard g' = dinv * (h1 @ W2) table is AllGathered
  so layer 2 can gather any src row locally.
- Pooling: PE-transpose h2^T tiles and matmul against per-tile graph
  one-hot indicators; per-core partial sums are combined on the host.
"""

import numpy as np
import ml_dtypes

# ---- problem constants (hardcoded per the harness contract) ----
N_NODES = 100000
N_EDGES = 1600000
N_GRAPHS = 512
IN_CH = 128
HID_CH = 128
OUT_CH = 64
NCORES = 8

# Optional profiling knob for the local test harness (ignored by grading).
PROFILE = {"enable": False, "tmpdir": None, "exec_time_ns": None}
BACKEND = "hw"  # "hw" | "sim" (sim is for small-scale testing only)

P = 128          # partitions / edge-block size
CHUNK = 25000    # gather-table chunk rows (int16 idx limit)
CALL = 1024      # idxs per dma_gather call (ucode limit)
BANK_D = 512     # dsts per PSUM bank (fp32 free dim)
SUP_BANKS = 3    # presum banks per superpass
SLICED_AG = False  # overlap per-superpass AllGather slices with layer 1
OH_MODE = "dve"    # "dve" (is_equal on DVE) | "fp8" (host-precomputed, streamed)


def _roundup(v, m):
    return (v + m - 1) // m * m


def _host_prepare(x, edge_index, batch, W1, b1, W2, b2):
    N, E, G = N_NODES, N_EDGES, N_GRAPHS
    SH = N // NCORES
    src = np.asarray(edge_index[0], dtype=np.int64)
    dst = np.asarray(edge_index[1], dtype=np.int64)
    batch = np.asarray(batch, dtype=np.int64)

    deg = np.bincount(dst, minlength=N).astype(np.float64) + 1.0
    dinv = (1.0 / np.sqrt(deg)).astype(np.float32)

    n_quarters = _roundup(SH, P) // P                       # 98
    sup_q = SUP_BANKS * (BANK_D // P)                       # 20 quarters / superpass
    n_sup = _roundup(n_quarters, sup_q) // sup_q            # 5
    n_chunks = _roundup(N, CHUNK) // CHUNK                  # 4

    # slice-major node permutation: node's row in the gather tables is
    # slice_off[s0] + c0*sz[s0] + local, where s0 is the superpass (of the
    # node as a dst) and c0 its core. AllGather of per-superpass g' slices
    # then lands contiguously in the permuted table.
    sup_d = sup_q * P                                       # dsts per superpass
    sz = [min(sup_d, SH - s * sup_d) for s in range(n_sup)]
    slice_off = np.concatenate([[0], np.cumsum([NCORES * z for z in sz])])
    node = np.arange(N)
    if SLICED_AG:
        c0 = node // SH
        loc = node - c0 * SH
        s0 = np.minimum(loc // sup_d, n_sup - 1)
        perm = (slice_off[s0] + c0 * np.asarray(sz)[s0]
                + (loc - s0 * sup_d)).astype(np.int64)      # node -> table row
    else:
        perm = node.copy()
    inv = np.empty(N, np.int64)
    inv[perm] = node

    xs_n = (np.asarray(x, np.float32) * dinv[:, None]).astype(ml_dtypes.bfloat16)
    xs = np.ascontiguousarray(xs_n[inv])                    # permuted table

    core_of = dst // SH
    q_of = (dst - core_of * SH) // P
    psrc = perm[src]
    k_of = psrc // CHUNK

    # group sizes: (q, k) -> padded max over cores
    counts = np.zeros((NCORES, n_quarters, n_chunks), np.int64)
    np.add.at(counts, (core_of, q_of, k_of), 1)
    gsz = _roundup(np.max(counts, axis=0), P)               # [n_quarters, n_chunks]

    # order edges per core by (superpass, chunk, quarter, src)
    sup_of = q_of // sup_q
    order = np.lexsort((psrc, q_of, k_of, sup_of, core_of))
    src_s, dst_s, core_s = psrc[order], dst[order], core_of[order]
    counts_s = counts  # [core, q, k]

    # stream layout (same for all cores): for s, for k, for q in s: gsz[q,k]
    stream_groups = []   # (s, k, q, size)
    for s in range(n_sup):
        qlo, qhi = s * sup_q, min((s + 1) * sup_q, n_quarters)
        for k in range(n_chunks):
            for q in range(qlo, qhi):
                stream_groups.append((s, k, q, int(gsz[q, k])))
    T = sum(g[3] for g in stream_groups)                    # padded edges/core

    idx16 = np.zeros((NCORES, T), np.int16)
    ids = np.full((NCORES, T), -1.0, np.float32)

    # per-core fill: edges are sorted by (core, sup, k, q, src) already
    core_pos = np.searchsorted(core_s, np.arange(NCORES + 1))
    for c in range(NCORES):
        lo = core_pos[c]
        off = 0
        ptr = lo
        for (s, k, q, size) in stream_groups:
            n = int(counts_s[c, q, k])
            sl = slice(ptr, ptr + n)
            idx16[c, off:off + n] = (src_s[sl] - k * CHUNK).astype(np.int16)
            ids[c, off:off + n] = (dst_s[sl] - (c * SH + q * P)).astype(np.float32)
            ptr += n
            off += size
        assert ptr == core_pos[c + 1], (c, ptr, core_pos[c + 1])
        assert off == T

    # wrap idx into [128, T/16] (16-partition groups, replicated x8)
    idx_w = np.zeros((NCORES, 128, T // 16), np.int16)
    w = idx16.reshape(NCORES, T // 16, 16).transpose(0, 2, 1)
    for g in range(8):
        idx_w[:, g * 16:(g + 1) * 16, :] = w
    # ids into [128, T/128] (edge i at [i%128, i//128])
    ids_w = ids.reshape(NCORES, T // P, P).transpose(0, 2, 1).astype(ml_dtypes.bfloat16)
    if OH_MODE == "fp8":
        ohv = np.zeros((NCORES, T // P, P, P), ml_dtypes.float8_e4m3)
        idv = ids.reshape(NCORES, T // P, P).astype(np.int64)
        bidx, pidx = np.meshgrid(np.arange(T // P), np.arange(P), indexing="ij")
        for c in range(NCORES):
            m = idv[c] >= 0
            ohv[c][bidx[m], pidx[m], idv[c][m]] = 1.0

    # block metadata (identical across cores)
    blocks = []   # (bank_id_in_sup, qib, s)
    calls = []    # (s, k, col0, nidx, blk0, nblk)
    boff = 0
    gi = 0
    while gi < len(stream_groups):
        s, k, q, size = stream_groups[gi]
        # merge consecutive groups with same (s, k) into one run
        run = size
        gj = gi + 1
        while gj < len(stream_groups) and stream_groups[gj][0] == s and stream_groups[gj][1] == k:
            run += stream_groups[gj][3]
            gj += 1
        # blocks for this run
        for (s2, k2, q2, size2) in stream_groups[gi:gj]:
            for _ in range(size2 // P):
                qloc = q2 - s2 * sup_q
                blocks.append((qloc // (BANK_D // P), qloc % (BANK_D // P), s2))
        # calls for this run
        done = 0
        while done < run:
            n = min(CALL, run - done)
            calls.append((s, k, (boff + done) // 16, n, (boff + done) // P, n // P))
            done += n
        boff += run
        gi = gj
    assert len(blocks) == T // P

    # first/last-touch per (s, bank, quarter) region
    last_touch = {}
    first_touch = {}
    for bi, (b, qib, s) in enumerate(blocks):
        last_touch[(s, b, qib)] = bi
        first_touch.setdefault((s, b, qib), bi)
    stop_set = set(last_touch.values())
    start_set = set(first_touch.values())
    # every quarter region must be touched, else PSUM garbage leaks
    n_regions = len(first_touch)
    assert n_regions == len({(s, b, q) for (b, q, s) in blocks})

    # per-superpass dst ranges
    sup_dst = []
    for s in range(n_sup):
        d0 = s * sup_q * P
        d1 = min((s + 1) * sup_q * P, SH)
        sup_dst.append((d0, d1))

    # self-term tables and dinv layouts, per core
    x_f32 = np.asarray(x, np.float32)
    sxT = np.zeros((NCORES, 128, n_quarters * P), ml_dtypes.bfloat16)
    dinv_bcast = np.zeros((NCORES, 128, n_quarters * P), ml_dtypes.bfloat16)
    dinvP = np.zeros((NCORES, 128, n_quarters), np.float32)
    for c in range(NCORES):
        sh = slice(c * SH, (c + 1) * SH)
        xsv = (x_f32[sh] * dinv[sh, None])                      # [SH, 128]
        sxT[c, :, :SH] = xsv.T.astype(ml_dtypes.bfloat16)
        dinv_bcast[c, :, :SH] = np.broadcast_to(
            dinv[sh][None, :], (128, SH)).astype(ml_dtypes.bfloat16)
        dP = np.zeros(n_quarters * P, np.float32)
        dP[:SH] = dinv[sh]
        dinvP[c] = dP.reshape(n_quarters, P).T

    iota = np.broadcast_to(np.arange(P, dtype=np.float32)[None, :], (P, P)).astype(
        ml_dtypes.bfloat16)

    # pooling indicators
    g_start = np.zeros(NCORES, np.int64)
    g_cnt = np.zeros(NCORES, np.int64)
    for c in range(NCORES):
        bsl = batch[c * SH:(c + 1) * SH]
        g_start[c] = bsl[0]
        g_cnt[c] = bsl[-1] - bsl[0] + 1
    NG = int(_roundup(int(g_cnt.max()), 8))
    poolind = np.zeros((NCORES, n_quarters, 128, NG), ml_dtypes.bfloat16)
    for c in range(NCORES):
        bsl = batch[c * SH:(c + 1) * SH] - g_start[c]
        node = np.arange(SH)
        poolind[c, node // P, node % P, bsl] = 1.0

    cnts = np.bincount(batch, minlength=G).astype(np.float32)
    inv_cnt = 1.0 / np.maximum(cnts, 1.0)

    meta = dict(SH=SH, n_quarters=n_quarters, sup_q=sup_q, n_sup=n_sup,
                n_chunks=n_chunks, T=T, blocks=blocks, calls=calls,
                stop_set=stop_set, start_set=start_set, sup_dst=sup_dst, NG=NG,
                sz=sz, slice_off=[int(v) for v in slice_off])
    per_core = []
    for c in range(NCORES):
        per_core.append({
            "xs": np.ascontiguousarray(xs),
            "idx": np.ascontiguousarray(idx_w[c]),
            **({"ids": np.ascontiguousarray(ids_w[c]), "iota": iota}
               if OH_MODE == "dve" else
               {"oh": np.ascontiguousarray(
                   ohv[c].transpose(1, 0, 2).reshape(P, T // P * P))}),
            "sxT": np.ascontiguousarray(sxT[c]),
            "dinv_bcast": np.ascontiguousarray(dinv_bcast[c]),
            "dinvP": np.ascontiguousarray(dinvP[c]),
            "poolind": np.ascontiguousarray(
                poolind[c].reshape(n_quarters * 128, NG)),
            "W1": np.asarray(W1, np.float32).astype(ml_dtypes.bfloat16),
            "W2": np.asarray(W2, np.float32).astype(ml_dtypes.bfloat16),
            "b1": np.asarray(b1, np.float32).reshape(HID_CH, 1),
            "b2": np.asarray(b2, np.float32).reshape(OUT_CH, 1),
        })
    host = dict(g_start=g_start, g_cnt=g_cnt, inv_cnt=inv_cnt)
    return meta, per_core, host


def _patch_swdge_lane_assignment():
    """Make Tile's DMASW semaphore-lane choice queue-aware.

    TileClockTick._assign_tick hands Pool-engine DMA instructions DMASW
    lanes round-robin, ignoring queue_num. Two SWDGE queues sharing one
    lane-semaphore can complete out of order, so a consumer's wait_ge can
    fire before its producer's DMA landed. Pin lanes 2q/2q+1 to queue q.
    """
    from concourse import tile_sem_assignment as tsa
    if getattr(tsa, "_queue_lane_patch", False):
        return
    tsa._queue_lane_patch = True
    orig = tsa.TileClockTick._assign_tick
    import concourse.mybir as mybir

    def patched(self, inst):
        qn = getattr(inst, "queue_num", None)
        if (qn is not None and isinstance(inst, tsa.DMAInst)
                and inst.engine == mybir.EngineType.Pool):
            tog = getattr(self, "_qtoggle", None)
            if tog is None:
                tog = self._qtoggle = {}
            t = tog.get(qn, 0)
            tog[qn] = t ^ 1
            self.next_sw_dma_idx = (2 * qn + t) % self.swdge_sem_count
        return orig(self, inst)

    tsa.TileClockTick._assign_tick = patched


def _build_program(meta):
    import concourse.bacc as bacc
    import concourse.bass as bass
    import concourse.mybir as mybir
    import concourse.tile as tile
    from concourse.masks import make_identity

    _patch_swdge_lane_assignment()

    SH = meta["SH"]
    nq = meta["n_quarters"]
    n_sup = meta["n_sup"]
    sup_q = meta["sup_q"]
    n_chunks = meta["n_chunks"]
    T = meta["T"]
    blocks = meta["blocks"]
    calls = meta["calls"]
    stop_set = meta["stop_set"]
    start_set = meta["start_set"]
    sup_dst = meta["sup_dst"]
    NG = meta["NG"]
    sz = meta["sz"]
    slice_off = meta["slice_off"]
    NQP = nq * P          # padded shard nodes (12544)

    dt32 = mybir.dt.float32
    dtb = mybir.dt.bfloat16

    nc = bacc.Bacc("TRN2", target_bir_lowering=False, debug=False,
                   num_devices=NCORES, num_swdge_queues=4)

    xs_in = nc.declare_dram_parameter("xs", [N_NODES, IN_CH], dtb, isOutput=False)
    idx_in = nc.declare_dram_parameter("idx", [128, T // 16], mybir.dt.int16, isOutput=False)
    if OH_MODE == "dve":
        ids_in = nc.declare_dram_parameter("ids", [128, T // P], dtb, isOutput=False)
        iota_in = nc.declare_dram_parameter("iota", [P, P], dtb, isOutput=False)
    else:
        oh_in = nc.declare_dram_parameter("oh", [P, T // P * P], mybir.dt.float8e4,
                                          isOutput=False)
    sxT_in = nc.declare_dram_parameter("sxT", [128, NQP], dtb, isOutput=False)
    dbc_in = nc.declare_dram_parameter("dinv_bcast", [128, NQP], dtb, isOutput=False)
    dP_in = nc.declare_dram_parameter("dinvP", [128, nq], dt32, isOutput=False)
    pind_in = nc.declare_dram_parameter("poolind", [nq * 128, NG], dtb, isOutput=False)
    W1_in = nc.declare_dram_parameter("W1", [IN_CH, HID_CH], dtb, isOutput=False)
    W2_in = nc.declare_dram_parameter("W2", [HID_CH, OUT_CH], dtb, isOutput=False)
    b1_in = nc.declare_dram_parameter("b1", [HID_CH, 1], dt32, isOutput=False)
    b2_in = nc.declare_dram_parameter("b2", [OUT_CH, 1], dt32, isOutput=False)
    pooled_out = nc.declare_dram_parameter("pooled", [NG, OUT_CH], dt32, isOutput=True)

    with tile.TileContext(nc) as tc:
        with (
            tc.tile_pool(name="const", bufs=1) as constp,
            tc.tile_pool(name="gout", bufs=20) as goutp,
            tc.tile_pool(name="ohp", bufs=10) as ohp,
            tc.tile_pool(name="evac", bufs=3) as evacp,
            tc.tile_pool(name="h1p", bufs=2) as h1p,
            tc.tile_pool(name="small", bufs=3) as smallp,
            tc.tile_pool(name="pres", bufs=2 * SUP_BANKS, space="PSUM") as presp,
            tc.tile_pool(name="psw", bufs=1, space="PSUM") as pswp,
            tc.tile_pool(name="poolacc", bufs=1, space="PSUM") as poolaccp,
            tc.tile_pool(name="dram", bufs=1, space="DRAM") as dramp,
        ):
            # ---- load constants ----
            idx_t = constp.tile([128, T // 16], mybir.dt.int16)
            if OH_MODE == "dve":
                ids_t = constp.tile([128, T // P], dtb)
                iota = constp.tile([P, P], dtb)
            sxT = constp.tile([128, NQP], dtb)
            dbc = constp.tile([128, NQP], dtb)
            dP = constp.tile([128, nq], dt32)
            W1t = constp.tile([IN_CH, HID_CH], dtb)
            W2t = constp.tile([HID_CH, OUT_CH], dtb)
            b1t = constp.tile([HID_CH, 1], dt32)
            b2t = constp.tile([OUT_CH, 1], dt32)
            ident = constp.tile([P, P], dt32)
            sg2T = constp.tile([64, NQP], dtb)
            nc.sync.dma_start(out=idx_t[:], in_=idx_in[:, :])
            if OH_MODE == "dve":
                nc.sync.dma_start(out=ids_t[:], in_=ids_in[:, :])
                nc.sync.dma_start(out=iota[:], in_=iota_in[:, :])
            nc.sync.dma_start(out=sxT[:], in_=sxT_in[:, :])
            nc.sync.dma_start(out=dbc[:], in_=dbc_in[:, :])
            nc.sync.dma_start(out=dP[:], in_=dP_in[:, :])
            nc.sync.dma_start(out=W1t[:], in_=W1_in[:, :])
            nc.sync.dma_start(out=W2t[:], in_=W2_in[:, :])
            nc.sync.dma_start(out=b1t[:], in_=b1_in[:, :])
            nc.sync.dma_start(out=b2t[:], in_=b2_in[:, :])
            make_identity(nc, ident[:])

            g_shard = dramp.tile([SH, HID_CH], dtb)      # g' rows (padded to 128 bf16)
            g_full = nc.dram_tensor("g_full_sh", [N_NODES, HID_CH], dtb,
                                    addr_space="Shared")

            def layer_presum(table_ap, lhs_cols, out_parts):
                """One message-passing layer's presum. Returns per-superpass
                callback hook: dict s -> list of psum bank tiles."""
                sup_banks = {}
                call_list = [cl for cl in calls]
                # allocate + zero psum banks lazily per superpass
                cur_s = -1
                for ci, (s, k, col0, nidx, blk0, nblk) in enumerate(call_list):
                    if s != cur_s:
                        cur_s = s
                        d0, d1 = sup_dst[s]
                        nb = (_roundup(d1 - d0, BANK_D)) // BANK_D
                        tiles = []
                        for b in range(nb):
                            pb = presp.tile([out_parts, BANK_D], dt32, space="PSUM",
                                            tag="presum")
                            nc.vector.memset(pb[:], 0.0)
                            tiles.append(pb)
                        sup_banks[s] = tiles
                        yield ("sup_start", s, tiles)
                    g = goutp.tile([128, CALL // P, 128], dtb, tag="gout")
                    nc.gpsimd.dma_gather(
                        out_ap=g[:, :nblk, :],
                        in_ap=table_ap[k * CHUNK:min((k + 1) * CHUNK, N_NODES), :],
                        idxs_ap=idx_t[:, col0:col0 + nidx // 16],
                        num_idxs=nidx, num_idxs_reg=nidx, elem_size=128,
                        queue_num=ci % 4)
                    if OH_MODE == "dve":
                        oh = ohp.tile([128, CALL // P, 128], dtb, tag="oh")
                        in0 = ids_t[:, blk0:blk0 + nblk, None].to_broadcast(
                            [128, nblk, 128])
                        ap1 = iota[:]
                        in1 = bass.AP(ap1.tensor, ap1.offset,
                                      [ap1.ap[0], [0, nblk], ap1.ap[1]])
                        nc.vector.tensor_tensor(out=oh[:, :nblk, :], in0=in0,
                                                in1=in1,
                                                op=mybir.AluOpType.is_equal)
                    else:
                        oh = ohp.tile([128, CALL // P, 128], mybir.dt.float8e4,
                                      tag="oh")
                        nc.sync.dma_start(
                            out=oh[:, :nblk, :],
                            in_=oh_in[:, blk0 * P:(blk0 + nblk) * P].rearrange(
                                "p (b c) -> p b c", c=P))
                    for j in range(nblk):
                        bi = blk0 + j
                        bank, qib, s2 = blocks[bi]
                        pb = sup_banks[s2][bank]
                        nc.tensor.matmul(
                            out=pb[:, qib * P:(qib + 1) * P],
                            lhsT=g[:, j, :lhs_cols],
                            rhs=oh[:, j, :],
                            start=False, stop=(bi in stop_set),
                            skip_group_check=True)
                    # end of superpass?
                    nxt = call_list[ci + 1][0] if ci + 1 < len(call_list) else None
                    if nxt != s:
                        yield ("sup_end", s, sup_banks[s])

            # ---------------- layer 1 ----------------
            for ev, s, tiles in layer_presum(xs_in[:], 128, 128):
                if SLICED_AG and ev == "sup_start" and s > 0:
                    # allgather the previous superpass's g' slice while this
                    # superpass's gathers run
                    sp = s - 1
                    r0 = sp * sup_q * P
                    nc.gpsimd.collective_compute(
                        "AllGather", mybir.AluOpType.bypass,
                        replica_groups=[list(range(NCORES))],
                        ins=[g_shard[r0:r0 + sz[sp], :]],
                        outs=[g_full[slice_off[sp]:slice_off[sp + 1], :]],
                    )
                if ev != "sup_end":
                    continue
                d0, d1 = sup_dst[s]
                for b, pb in enumerate(tiles):
                    c0 = d0 + b * BANK_D          # dst offset in shard
                    w = min(BANK_D, NQP - c0)
                    cols = slice(c0, c0 + w)
                    xt = evacp.tile([128, BANK_D], dtb, tag="xt")
                    nc.vector.tensor_tensor(out=xt[:, :w], in0=pb[:, :w],
                                            in1=sxT[:, cols],
                                            op=mybir.AluOpType.add)
                    h1pre = pswp.tile([128, BANK_D], dt32, space="PSUM", tag="w")
                    nc.tensor.matmul(out=h1pre[:, :w], lhsT=W1t[:], rhs=xt[:, :w],
                                     start=True, stop=True)
                    tmp = evacp.tile([128, BANK_D], dt32, tag="tmp")
                    nc.vector.tensor_tensor(out=tmp[:, :w], in0=h1pre[:, :w],
                                            in1=dbc[:, cols],
                                            op=mybir.AluOpType.mult)
                    h1T = h1p.tile([128, BANK_D], dtb, tag="h1T")
                    nc.scalar.activation(h1T[:, :w], tmp[:, :w],
                                         mybir.ActivationFunctionType.Relu,
                                         bias=b1t[:, 0:1])
                    # gT = (h1 W2)^T for the layer-2 self term
                    gT = pswp.tile([64, BANK_D], dt32, space="PSUM", tag="w")
                    nc.tensor.matmul(out=gT[:, :w], lhsT=W2t[:], rhs=h1T[:, :w],
                                     start=True, stop=True)
                    nc.vector.tensor_tensor(out=sg2T[:, cols], in0=gT[:, :w],
                                            in1=dbc[:64, cols],
                                            op=mybir.AluOpType.mult)
                    # g' table rows per quarter
                    for qib in range(_roundup(w, P) // P):
                        t_global = (c0 + qib * P) // P
                        gp = pswp.tile([128, 64], dt32, space="PSUM", tag="w")
                        nc.tensor.matmul(out=gp[:],
                                         lhsT=h1T[:, qib * P:(qib + 1) * P],
                                         rhs=W2t[:], start=True, stop=True)
                        gst = smallp.tile([128, 128], dtb, tag="gst")
                        nc.vector.memset(gst[:], 0.0)
                        nc.vector.tensor_scalar_mul(gst[:, :64], gp[:],
                                                    dP[:, t_global:t_global + 1])
                        r0 = t_global * P
                        rows = min(P, SH - r0)
                        nc.sync.dma_start(out=g_shard[r0:r0 + rows, :],
                                          in_=gst[:rows, :])


            # ---------------- layer 2 + pooling ----------------
            if SLICED_AG:
                sp = n_sup - 1
                r0 = sp * sup_q * P
                nc.gpsimd.collective_compute(
                    "AllGather", mybir.AluOpType.bypass,
                    replica_groups=[list(range(NCORES))],
                    ins=[g_shard[r0:r0 + sz[sp], :]],
                    outs=[g_full[slice_off[sp]:slice_off[sp + 1], :]],
                )
            else:
                nc.gpsimd.collective_compute(
                    "AllGather", mybir.AluOpType.bypass,
                    replica_groups=[list(range(NCORES))],
                    ins=[g_shard.opt()],
                    outs=[g_full[:, :]],
                )

            pool_acc = poolaccp.tile([_roundup(NG, 8), 64], dt32, space="PSUM",
                                     tag="pool")
            n_tiles_done = 0
            for ev, s, tiles in layer_presum(g_full[:], 64, 64):
                if ev != "sup_end":
                    continue
                d0, d1 = sup_dst[s]
                for b, pb in enumerate(tiles):
                    c0 = d0 + b * BANK_D
                    w = min(BANK_D, NQP - c0)
                    cols = slice(c0, c0 + w)
                    tmp = evacp.tile([128, BANK_D], dt32, tag="tmp")
                    nc.vector.tensor_tensor(out=tmp[:64, :w], in0=pb[:64, :w],
                                            in1=sg2T[:, cols],
                                            op=mybir.AluOpType.add)
                    h2T = evacp.tile([128, BANK_D], dt32, tag="h2T")
                    nc.vector.memset(h2T[:], 0.0)
                    nc.vector.tensor_tensor(out=h2T[:64, :w], in0=tmp[:64, :w],
                                            in1=dbc[:64, cols],
                                            op=mybir.AluOpType.mult)
                    nc.vector.tensor_scalar_add(h2T[:64, :w], h2T[:64, :w],
                                                b2t[:, 0:1])
                    for qib in range(_roundup(w, P) // P):
                        t_global = (c0 + qib * P) // P
                        trp = pswp.tile([128, 128], dt32, space="PSUM", tag="w")
                        nc.tensor.transpose(out=trp[:],
                                            in_=h2T[:, qib * P:(qib + 1) * P],
                                            identity=ident[:])
                        h2e = smallp.tile([128, 64], dtb, tag="h2e")
                        nc.vector.tensor_copy(out=h2e[:], in_=trp[:, :64])
                        pind = smallp.tile([128, NG], dtb, tag="pind")
                        nc.sync.dma_start(
                            out=pind[:],
                            in_=pind_in[t_global * 128:(t_global + 1) * 128, :])
                        nc.tensor.matmul(out=pool_acc[:NG, :], lhsT=pind[:],
                                         rhs=h2e[:],
                                         start=(n_tiles_done == 0),
                                         stop=(n_tiles_done == nq - 1),
                                         skip_group_check=True)
                        n_tiles_done += 1
            pe = smallp.tile([_roundup(NG, 8), 64], dt32, tag="pe")
            nc.vector.tensor_copy(out=pe[:NG, :], in_=pool_acc[:NG, :])
            nc.sync.dma_start(out=pooled_out[:, :], in_=pe[:NG, :])

    nc.compile()
    return nc


def kernel(x, edge_index, batch, W1, b1, W2, b2):
    meta, per_core, host = _host_prepare(x, edge_index, batch, W1, b1, W2, b2)
    nc = _build_program(meta)

    in_maps = [per_core[c] for c in range(NCORES)]
    if BACKEND == "sim":
        from concourse.bass_interp import MultiCoreSim
        sim = MultiCoreSim(nc, num_cores=NCORES, trace=False)
        for c in range(NCORES):
            for name, arr in in_maps[c].items():
                sim.cores[c].tensor(name)[:] = arr
        sim.simulate()
        parts = [np.asarray(sim.cores[c].tensor("pooled")) for c in range(NCORES)]
    else:
        from concourse.bass_utils import run_bass_kernel_spmd
        r = run_bass_kernel_spmd(nc, in_maps, list(range(NCORES)))
        PROFILE["exec_time_ns"] = r.exec_time_ns
        parts = [np.asarray(r.results[c]["pooled"]) for c in range(NCORES)]

    g_start, g_cnt = host["g_start"], host["g_cnt"]
    full = np.zeros((N_GRAPHS, OUT_CH), np.float32)
    for c in range(NCORES):
        n = int(g_cnt[c])
        full[g_start[c]:g_start[c] + n] += parts[c][:n].astype(np.float32)
    out = full * host["inv_cnt"][:, None]
    return out.astype(np.float32)

